# revision 11
# baseline (speedup 1.0000x reference)
"""GNN message-passing (GINEConv-style) distributed Bass kernel for 8 TRN2 cores.

Self-contained: takes full inputs, shards internally, runs one SPMD NEFF on
cores 0-7, gathers full outputs.

Architecture (v2):
  One dst-sorted edge stream per core (core owns node range of 6272 = 49x128;
  edges bucketed by dst owner, sorted by dst, segmented per 128-node tile,
  sub-segmented by src<32768 (lo/hi gather tables), chunk-padded to 128).
  Phase 1: bulk dma_gather of nfeat16[src] (non-transposed, <=512/call),
    msg = relu(gather + pre-permuted efeat), scatter-add via one-hot matmul
    into per-node-tile PSUM (aggT[d,n]); epilogue computes x (f32), y2 tile
    (x @ W2, bf16), x shard (bf16), and the full node output branch.
  Phase 2: AllGather of x shards -> x_all [50176, 128] bf16.
  Phase 3 (same stream): bulk dma_gather of x_all[src] TRANSPOSED [d, e];
    x_dst term via onehotT matmul against the local y2 tile (no gather);
    3-term FFN accumulation + proj head; outputs unpermuted on host.
"""
import sys

sys.path.insert(0, "/opt/trn_rl_repo")

import numpy as np
import ml_dtypes

import concourse.bass as bass
import concourse.mybir as mybir
import concourse.tile as tile
from concourse import bacc
from concourse import bass_utils

bf16 = ml_dtypes.bfloat16
f32 = np.float32

N, E, D, K = 50000, 800000, 128, 32
NC = 8
NSH = 6272          # nodes per core (49 * 128)
PADN = NC * NSH     # 50176
NT = NSH // 128     # 49 node tiles per core
HLIM = 32768        # int16 gather-index limit; src<HLIM -> lo table

_cache = {}


def _build_schedule(src, dst):
    """Shared SPMD schedule. Edges keyed by (dst_tile=dst//128, half=src>=HLIM).
    Per (tile, half): capacity = max over cores of count, padded to 128.
    Returns dict with per-chunk tile/half arrays and gather-call list."""
    core = dst // NSH
    gtile = dst // 128          # global tile id: core = gtile//NT, local t = gtile%NT
    half = (src >= HLIM).astype(np.int64)
    key = (gtile % NT) * 2 + half               # local (tile, half) in [0, 98)
    cnt = np.zeros((NC, NT * 2), np.int64)
    np.add.at(cnt, (core, key), 1)
    cap = cnt.max(axis=0)                        # [98]
    chunks_th = np.maximum(-(-cap // 128), (cap > 0).astype(np.int64))
    # force >=1 chunk for each tile's lo part so every tile exists
    chunks_th[0::2] = np.maximum(chunks_th[0::2], 1)
    C1 = int(chunks_th.sum())
    # flat chunk arrays
    chunk_tile = []
    chunk_half = []
    for t in range(NT):
        chunk_tile += [t] * int(chunks_th[2 * t]) + [t] * int(chunks_th[2 * t + 1])
        chunk_half += [0] * int(chunks_th[2 * t]) + [1] * int(chunks_th[2 * t + 1])
    chunk_tile = np.array(chunk_tile, np.int64)
    chunk_half = np.array(chunk_half, np.int64)
    tile_first = np.zeros(NT, np.int64)
    tile_last = np.zeros(NT, np.int64)
    for t in range(NT):
        w = np.where(chunk_tile == t)[0]
        tile_first[t] = w[0]
        tile_last[t] = w[-1]
    # gather calls: maximal runs of <=4 same-half chunks
    calls = []   # (chunk_start, n_chunks, half)
    c = 0
    while c < C1:
        h = chunk_half[c]
        k = 1
        while k < 4 and c + k < C1 and chunk_half[c + k] == h:
            k += 1
        calls.append((c, k, int(h)))
        c += k
    # chunk slot offsets per (tile, half)
    th_chunk_off = np.zeros(NT * 2, np.int64)
    off = 0
    for t in range(NT):
        th_chunk_off[2 * t] = off
        off += chunks_th[2 * t]
        th_chunk_off[2 * t + 1] = off
        off += chunks_th[2 * t + 1]
    return dict(C1=C1, chunk_tile=chunk_tile, chunk_half=chunk_half,
                tile_first=tile_first, tile_last=tile_last, calls=calls,
                th_chunk_off=th_chunk_off)


def _preprocess(inputs):
    src = np.asarray(inputs["src"], np.int32)
    dst = np.asarray(inputs["dst"], np.int32)
    nfeat = np.asarray(inputs["nfeat"], f32)
    efeat = np.asarray(inputs["efeat"], f32)
    ndist = np.asarray(inputs["ndist"], f32)
    edist = np.asarray(inputs["edist"], f32)

    sch = _build_schedule(src, dst)
    C1 = sch["C1"]
    NS = C1 * 128                      # total slots per core
    th_off = sch["th_chunk_off"] * 128   # slot offset per (tile, half)

    # order: sort globally by (core, tile, half, dst)
    gtile = dst // 128
    half = (src >= HLIM).astype(np.int64)
    sortkey = gtile * 2 + half
    order = np.argsort(sortkey, kind="stable")

    ef16 = efeat.astype(bf16)
    nf16 = np.zeros((PADN, D), bf16)
    nf16[:N] = nfeat.astype(bf16)

    # wrapped gather-index column layout
    calls = sch["calls"]
    call_cols = [k * 8 for (_, k, _) in calls]
    GCOLS = int(np.sum(call_cols))
    call_col_off = np.concatenate([[0], np.cumsum(call_cols)])[:-1]

    in_maps = []
    perms = []
    for c in range(NC):
        sel = (dst[order] // NSH) == c
        o_c = order[sel]
        d_c = dst[o_c]
        s_c = src[o_c]
        key_c = ((d_c // 128) % NT) * 2 + (s_c >= HLIM)
        # rank within each (tile, half) group (o_c is sorted by key_c)
        kstart = np.searchsorted(key_c, np.arange(NT * 2))
        rank = np.arange(len(o_c)) - kstart[key_c]
        slot = th_off[key_c] + rank

        src_slot = np.zeros(NS, np.int32)
        dstloc_slot = np.full(NS, 200.0, f32)
        ef_slot = np.zeros((NS, D), bf16)
        edT_slot = np.zeros((NS, K), bf16)
        src_slot[slot] = s_c
        dstloc_slot[slot] = (d_c % 128).astype(f32)
        ef_slot[slot] = ef16[o_c]
        edT_slot[slot] = edist[o_c].astype(bf16)

        # gather indices, wrapped per call: partition 16 + i%16, col i//16
        gidx = np.zeros((128, GCOLS), np.int16)
        tok = src_slot - (src_slot >= HLIM) * HLIM
        for (c0, k, h), coff in zip(calls, call_col_off):
            v = tok[c0 * 128:(c0 + k) * 128]
            gidx[16:32, coff:coff + k * 8] = v.astype(np.int16).reshape(-1, 16).T

        m = {
            "gidx": gidx,
            "dstloc1": dstloc_slot.reshape(C1, 128).T.astype(bf16),
            "dstrow": dstloc_slot.astype(bf16)[None, :],
            "efp": ef_slot.reshape(C1, 128, D).transpose(1, 0, 2).reshape(128, NS),
            "edT3": edT_slot.T.copy(),
            "nf16": nf16,
            "identb": np.eye(128, dtype=bf16),
            "nfT": np.zeros((D, NSH), f32),
            "ndT": np.zeros((K, NSH), f32),
        }
        nlo, nhi = c * NSH, min((c + 1) * NSH, N)
        m["nfT"][:, : nhi - nlo] = nfeat[nlo:nhi].T
        m["ndT"][:, : nhi - nlo] = ndist[nlo:nhi].T

        m["ginW"] = np.asarray(inputs["gin_W"], f32)
        m["ginB"] = np.asarray(inputs["gin_b"], f32).reshape(D, 1)
        m["ndW"] = np.asarray(inputs["node_dist_W"], f32)
        m["ndB"] = np.asarray(inputs["node_dist_b"], f32).reshape(K, 1)
        m["nfW1"] = np.asarray(inputs["node_ffn_W"], f32)[:D]
        m["nfW2"] = np.asarray(inputs["node_ffn_W"], f32)[D:]
        m["nfB"] = np.asarray(inputs["node_ffn_b"], f32).reshape(D, 1)
        npW = np.zeros((D, 33), f32)
        npW[:, 0] = np.asarray(inputs["node_proj_W"], f32)[:, 0]
        npW[:, 32] = np.asarray(inputs["node_proj_W"], f32)[:, 1]
        m["npW"] = npW
        npB = np.zeros((33, 1), f32)
        npB[0, 0], npB[32, 0] = inputs["node_proj_b"][0], inputs["node_proj_b"][1]
        m["npB"] = npB
        m["eW1"] = np.asarray(inputs["edge_ffn_W"], f32)[:D].astype(bf16)
        m["eW2"] = np.asarray(inputs["edge_ffn_W"], f32)[D:2 * D].astype(bf16)
        m["eW3"] = np.asarray(inputs["edge_ffn_W"], f32)[2 * D:].astype(bf16)
        m["eB"] = np.asarray(inputs["edge_ffn_b"], f32).reshape(D, 1)
        m["edW"] = np.asarray(inputs["edge_dist_W"], f32).astype(bf16)
        m["edB"] = np.asarray(inputs["edge_dist_b"], f32).reshape(K, 1)
        epW = np.zeros((D, 33), f32)
        epW[:, 0] = np.asarray(inputs["edge_proj_W"], f32)[:, 0]
        epW[:, 32] = np.asarray(inputs["edge_proj_W"], f32)[:, 1]
        m["epW"] = epW.astype(bf16)
        epB = np.zeros((33, 1), f32)
        epB[0, 0], epB[32, 0] = inputs["edge_proj_b"][0], inputs["edge_proj_b"][1]
        m["epB"] = epB
        in_maps.append(m)
        perms.append((o_c, slot))

    return in_maps, sch, perms, (GCOLS, call_col_off)


def build(sch, gmeta):
    C1 = sch["C1"]
    NS = C1 * 128
    calls = sch["calls"]
    GCOLS, call_col_off = gmeta
    chunk_tile = sch["chunk_tile"]
    tile_first = sch["tile_first"]
    tile_last = sch["tile_last"]
    BF = mybir.dt.bfloat16
    F32 = mybir.dt.float32
    I16 = mybir.dt.int16
    AF = mybir.ActivationFunctionType
    OP = mybir.AluOpType

    nc = bacc.Bacc("TRN2", target_bir_lowering=False, debug=False, num_devices=NC)

    def din(name, shape, dt):
        return nc.dram_tensor(name, shape, dt, kind="ExternalInput").ap()

    t_gidx = din("gidx", [128, GCOLS], I16)
    t_dstloc1 = din("dstloc1", [128, C1], BF)
    t_dstrow = din("dstrow", [1, NS], BF)
    t_efp = din("efp", [128, NS], BF)
    t_nf16 = din("nf16", [PADN, D], BF)
    t_identb = din("identb", [128, 128], BF)
    t_nfT = din("nfT", [D, NSH], F32)
    t_ndT = din("ndT", [K, NSH], F32)
    t_edT3 = din("edT3", [K, NS], BF)
    wspec = [
        ("ginW", [D, D], F32), ("ginB", [D, 1], F32),
        ("ndW", [K, K], F32), ("ndB", [K, 1], F32),
        ("nfW1", [D, D], F32), ("nfW2", [K, D], F32), ("nfB", [D, 1], F32),
        ("npW", [D, 33], F32), ("npB", [33, 1], F32),
        ("eW1", [D, D], BF), ("eW2", [D, D], BF), ("eW3", [K, D], BF),
        ("eB", [D, 1], F32),
        ("edW", [K, K], BF), ("edB", [K, 1], F32),
        ("epW", [D, 33], BF), ("epB", [33, 1], F32),
    ]
    w = {name: din(name, shape, dt) for name, shape, dt in wspec}

    o_node = nc.dram_tensor("node_out", [2, NSH], F32, kind="ExternalOutput").ap()
    o_edge = nc.dram_tensor("edge_out", [2, NS], F32, kind="ExternalOutput").ap()

    with tile.TileContext(nc) as tc:
        with (
            tc.tile_pool(name="pers", bufs=1) as pp,
            tc.tile_pool(name="sb", bufs=1) as sb,
            tc.tile_pool(name="ps", bufs=1, space="PSUM") as ps,
            tc.tile_pool(name="drsh", bufs=1, space="DRAM") as drsh,
            tc.tile_pool(name="drall", bufs=1, space="DRAM") as drall,
        ):
            def load(src_ap, shape, dt, tag):
                t = pp.tile(shape, dt, tag=tag)
                nc.sync.dma_start(out=t[:], in_=src_ap)
                return t

            gidx_t = load(t_gidx[:, :], [128, GCOLS], I16, "p_gidx")
            dstloc1_t = load(t_dstloc1[:, :], [128, C1], BF, "p_dstloc1")
            nfT_t = load(t_nfT[:, :], [D, NSH], F32, "p_nfT")
            ndT_t = load(t_ndT[:, :], [K, NSH], F32, "p_ndT")
            wt = {name: load(w[name][:, :], shape, dt, "p_w_" + name)
                  for name, shape, dt in wspec}
            ident_b = load(t_identb[:, :], [128, 128], BF, "p_identb")

            # iota over j-chunk free dim (for phase-1 one-hot [e, n])
            iota_j = pp.tile([128, 4, D], BF)
            nc.gpsimd.iota(iota_j[:, :, :], pattern=[[0, 4], [1, D]], base=0,
                           channel_multiplier=0,
                           allow_small_or_imprecise_dtypes=True)
            # iota over partitions (for phase-3 onehotT [n, e])
            iota_p = pp.tile([128, 1], BF)
            nc.gpsimd.iota(iota_p[:, :], pattern=[[0, 1]], base=0,
                           channel_multiplier=1,
                           allow_small_or_imprecise_dtypes=True)
            ones_row = pp.tile([1, 128], BF)
            nc.vector.memset(ones_row[:], 1.0)

            node_stage_t = pp.tile([1, NSH], F32)
            node_stage_p = pp.tile([1, NSH], F32)
            y2_all = pp.tile([128, NT, D], BF)   # y2 per node tile

            x_shard = drsh.tile([NSH, D], BF)
            x_all = drall.tile([PADN, D], BF, addr_space="Shared")

            # ---------------- phase 1 ----------------
            psum_aggr = None
            for (c0, kch, h), coff in zip(calls, call_col_off):
                ncols = kch * 128
                ef_t = sb.tile([128, 4, D], BF, tag="ef", bufs=4)
                nc.sync.dma_start(
                    out=ef_t[:, :kch, :],
                    in_=t_efp[:, c0 * 128:(c0 + kch) * 128].rearrange(
                        "p (j d) -> p j d", j=kch))
                gx = sb.tile([128, 4, D], BF, tag="g1x", bufs=4)
                tab = t_nf16[:HLIM, :] if h == 0 else t_nf16[HLIM:, :]
                nc.gpsimd.dma_gather(
                    gx[:, :kch, :], tab, gidx_t[:, coff:coff + kch * 8],
                    ncols, ncols, D)
                msgr = sb.tile([128, 4, D], BF, tag="msgr", bufs=4)
                nc.vector.tensor_add(msgr[:, :kch, :], gx[:, :kch, :],
                                     ef_t[:, :kch, :])
                msg = sb.tile([128, 4, D], BF, tag="msg", bufs=4)
                nc.scalar.activation(msg[:, :kch, :], msgr[:, :kch, :], AF.Relu)
                onehot = sb.tile([128, 4, D], BF, tag="oh", bufs=4)
                nc.vector.tensor_tensor(
                    out=onehot[:, :kch, :],
                    in0=dstloc1_t[:, c0:c0 + kch, None].to_broadcast(
                        [128, kch, D]),
                    in1=iota_j[:, :kch, :],
                    op=OP.is_equal)
                for j in range(kch):
                    kchunk = c0 + j
                    t = int(chunk_tile[kchunk])
                    if kchunk == tile_first[t]:
                        psum_aggr = ps.tile([D, 128], F32, tag="aggT", bufs=2)
                    nc.tensor.matmul(
                        psum_aggr[:, :], lhsT=msg[:, j, :], rhs=onehot[:, j, :],
                        start=(kchunk == tile_first[t]),
                        stop=(kchunk == tile_last[t]))
                    if kchunk == tile_last[t]:
                        # ---------- node-tile epilogue ----------
                        cols = slice(t * 128, (t + 1) * 128)
                        AT = sb.tile([D, 128], F32, tag="AT", bufs=2)
                        nc.vector.tensor_add(AT[:], psum_aggr[:, :], nfT_t[:, cols])
                        xT_ps = ps.tile([D, 128], F32, tag="ep", bufs=2)
                        nc.tensor.matmul(xT_ps[:], lhsT=wt["ginW"][:], rhs=AT[:],
                                         start=True, stop=True)
                        xT = sb.tile([D, 128], F32, tag="xT", bufs=2)
                        nc.scalar.activation(xT[:], xT_ps[:], AF.Relu,
                                             bias=wt["ginB"][:, :])
                        xTb = sb.tile([D, 128], BF, tag="xTb", bufs=2)
                        nc.vector.tensor_copy(xTb[:], xT[:])
                        # y2 tile = x @ W2  -> [n, dout] bf16
                        y2_ps = ps.tile([128, D], F32, tag="ep", bufs=2)
                        nc.tensor.matmul(y2_ps[:], lhsT=xTb[:], rhs=wt["eW2"][:],
                                         start=True, stop=True)
                        nc.vector.tensor_copy(y2_all[:, t, :], y2_ps[:])
                        # x shard (row-major bf16) via transpose
                        x_ps = ps.tile([128, D], BF, tag="dbc", bufs=2)
                        nc.tensor.transpose(x_ps[:], xTb[:], ident_b[:])
                        x_sb = sb.tile([128, D], BF, tag="xsb", bufs=2)
                        nc.vector.tensor_copy(x_sb[:], x_ps[:])
                        nc.sync.dma_start(out=x_shard[cols, :], in_=x_sb[:])
                        # node branch
                        nd_ps = ps.tile([K, 128], F32, tag="ep", bufs=2)
                        nc.tensor.matmul(nd_ps[:], lhsT=wt["ndW"][:],
                                         rhs=ndT_t[:, cols], start=True, stop=True)
                        ndr = sb.tile([K, 128], F32, tag="ndr", bufs=2)
                        nc.scalar.activation(ndr[:], nd_ps[:], AF.Relu,
                                             bias=wt["ndB"][:, :])
                        nh_ps = ps.tile([D, 128], F32, tag="ep", bufs=2)
                        nc.tensor.matmul(nh_ps[:], lhsT=wt["nfW1"][:], rhs=xT[:],
                                         start=True, stop=False)
                        nc.tensor.matmul(nh_ps[:], lhsT=wt["nfW2"][:], rhs=ndr[:],
                                         start=False, stop=True)
                        nh = sb.tile([D, 128], F32, tag="nh", bufs=2)
                        nc.scalar.activation(nh[:], nh_ps[:], AF.Relu,
                                             bias=wt["nfB"][:, :])
                        no_ps = ps.tile([33, 128], F32, tag="ep", bufs=2)
                        nc.tensor.matmul(no_ps[:], lhsT=wt["npW"][:], rhs=nh[:],
                                         start=True, stop=True)
                        nob = sb.tile([33, 128], F32, tag="nob", bufs=2)
                        nc.scalar.activation(nob[:], no_ps[:], AF.Identity,
                                             bias=wt["npB"][:, :])
                        nc.vector.tensor_scalar(
                            out=node_stage_t[0:1, cols], in0=nob[0:1, :],
                            scalar1=1.0, scalar2=100.0, op0=OP.max, op1=OP.min)
                        nc.vector.tensor_copy(node_stage_p[0:1, cols],
                                              nob[32:33, :])

            nc.scalar.activation(node_stage_p[0:1, :], node_stage_p[0:1, :],
                                 AF.Sigmoid)
            nc.sync.dma_start(out=o_node[0:1, :], in_=node_stage_t[:, :])
            nc.sync.dma_start(out=o_node[1:2, :], in_=node_stage_p[:, :])

            # ---------------- phase 2: AllGather x ----------------
            nc.gpsimd.collective_compute(
                "AllGather", OP.bypass, replica_groups=[list(range(NC))],
                ins=[x_shard[:, :].opt()], outs=[x_all[:, :].opt()])

            # ---------------- phase 3 ----------------
            # output staging: windows of 2 calls
            stage_t = None
            stage_p = None
            win_start = 0
            win_cols = 0
            nwin = 0
            for ci, ((c0, kch, h), coff) in enumerate(zip(calls, call_col_off)):
                ncols = kch * 128
                if win_cols == 0:
                    stage_t = sb.tile([1, 1024], F32, tag="st", bufs=2)
                    stage_p = sb.tile([1, 1024], F32, tag="sp", bufs=2)
                    win_start = c0 * 128
                # gather x_all[src] transposed -> [d, e]
                xsT = sb.tile([128, 1, 512], BF, tag="xsT", bufs=4)
                tab = x_all[:HLIM, :] if h == 0 else x_all[HLIM:, :]
                nc.gpsimd.dma_gather(
                    xsT[:, :, :ncols], tab, gidx_t[:, coff:coff + kch * 8],
                    ncols, ncols, D, transpose=True)
                # edT strip
                edt = sb.tile([K, 512], BF, tag="edt", bufs=4)
                nc.sync.dma_start(out=edt[:, :ncols],
                                  in_=t_edT3[:, c0 * 128:c0 * 128 + ncols])
                ed_ps = ps.tile([K, 512], F32, tag="ep", bufs=2)
                nc.tensor.matmul(ed_ps[:, :ncols], lhsT=wt["edW"][:],
                                 rhs=edt[:, :ncols], start=True, stop=True)
                edr = sb.tile([K, 512], BF, tag="edr", bufs=2)
                nc.scalar.activation(edr[:, :ncols], ed_ps[:, :ncols], AF.Relu,
                                     bias=wt["edB"][:, :])
                # onehotT [n, e]: DMA-broadcast dst row across partitions + is_equal
                dbc = sb.tile([128, 512], BF, tag="dbcs", bufs=3)
                nc.sync.dma_start(
                    out=dbc[:, :ncols],
                    in_=t_dstrow[0:1, c0 * 128:c0 * 128 + ncols].to_broadcast(
                        [128, ncols]))
                ohT = sb.tile([128, 512], BF, tag="ohT", bufs=2)
                nc.vector.tensor_tensor(
                    out=ohT[:, :ncols],
                    in0=iota_p[:, 0:1].to_broadcast([128, ncols]),
                    in1=dbc[:, :ncols], op=OP.is_equal)
                # ehT accumulation [dout, e]
                ehT_ps = ps.tile([D, 512], F32, tag="ehT", bufs=2)
                nc.tensor.matmul(ehT_ps[:, :ncols], lhsT=wt["eW1"][:],
                                 rhs=xsT[:, 0, :ncols], start=True, stop=False)
                nc.tensor.matmul(ehT_ps[:, :ncols], lhsT=wt["eW3"][:],
                                 rhs=edr[:, :ncols], start=False, stop=False)
                for j in range(kch):
                    t = int(chunk_tile[c0 + j])
                    nc.tensor.matmul(
                        ehT_ps[:, j * 128:(j + 1) * 128],
                        lhsT=y2_all[:, t, :],
                        rhs=ohT[:, j * 128:(j + 1) * 128],
                        start=False, stop=(j == kch - 1))
                ehr = sb.tile([D, 512], BF, tag="ehr", bufs=2)
                nc.scalar.activation(ehr[:, :ncols], ehT_ps[:, :ncols], AF.Relu,
                                     bias=wt["eB"][:, :])
                eo_ps = ps.tile([33, 512], F32, tag="ep", bufs=2)
                nc.tensor.matmul(eo_ps[:, :ncols], lhsT=wt["epW"][:],
                                 rhs=ehr[:, :ncols], start=True, stop=True)
                eob = sb.tile([33, 512], F32, tag="eob", bufs=2)
                nc.vector.tensor_scalar_add(eob[:, :ncols], eo_ps[:, :ncols],
                                            wt["epB"][:, :])
                soff = c0 * 128 - win_start
                nc.vector.tensor_scalar(
                    out=stage_t[0:1, soff:soff + ncols], in0=eob[0:1, :ncols],
                    scalar1=1.0, scalar2=100.0, op0=OP.max, op1=OP.min)
                nc.vector.tensor_copy(stage_p[0:1, soff:soff + ncols],
                                      eob[32:33, :ncols])
                win_cols += ncols
                nwin += 1
                last = ci == len(calls) - 1
                if nwin == 2 or last:
                    nc.scalar.activation(stage_p[0:1, :win_cols],
                                         stage_p[0:1, :win_cols], AF.Sigmoid)
                    ocols = slice(win_start, win_start + win_cols)
                    nc.sync.dma_start(out=o_edge[0:1, ocols],
                                      in_=stage_t[0:1, :win_cols])
                    nc.sync.dma_start(out=o_edge[1:2, ocols],
                                      in_=stage_p[0:1, :win_cols])
                    win_cols = 0
                    nwin = 0

    nc.compile()
    return nc


def _run(inputs):
    in_maps, sch, perms, gmeta = _preprocess(inputs)
    key = (sch["C1"], tuple(sch["chunk_tile"][:64].tolist()),
           tuple(k for (_, k, _) in sch["calls"][:64]))
    if key not in _cache:
        _cache[key] = build(sch, gmeta)
    return _cache[key], in_maps, sch, perms


def _assemble(res, sch, perms):
    node_rows = np.concatenate([res.results[c]["node_out"] for c in range(NC)],
                               axis=1)  # [2, NC*NSH], core-major node order
    edge_t = np.empty(E, f32)
    edge_p = np.empty(E, f32)
    for c in range(NC):
        o_c, slot = perms[c]
        eo = res.results[c]["edge_out"]
        edge_t[o_c] = eo[0, slot]
        edge_p[o_c] = eo[1, slot]
    nt = node_rows[0, :N]
    npp = node_rows[1, :N]
    return (nt[:, None].astype(f32), npp[:, None].astype(f32),
            edge_t[:, None], edge_p[:, None])


def run_timed(inputs, tmpdir=None):
    import tempfile

    _install_ntff_hook()
    nc, in_maps, sch, perms = _run(inputs)
    if tmpdir is None:
        tmpdir = tempfile.mkdtemp(prefix="gnn_trace_")
    res = bass_utils.run_bass_kernel_spmd(
        nc, in_maps, core_ids=list(range(NC)), trace=True, tmpdir=tmpdir)
    return res.exec_time_ns, tmpdir, _assemble(res, sch, perms)


def kernel(**inputs):
    nc, in_maps, sch, perms = _run(inputs)
    res = bass_utils.run_bass_kernel_spmd(nc, in_maps, core_ids=list(range(NC)))
    return _assemble(res, sch, perms)


def _install_ntff_hook():
    """Register the ctypes NTFF profiling hook (the image's antenv lacks it)."""
    import antenv

    p = "/opt/trn_rl_repo/antenv"
    if p not in list(antenv.__path__):
        antenv.__path__.append(p)
    from antenv import axon_hooks

    if axon_hooks.get_axon_ntff_profile_hook() is not None:
        return
    import contextlib
    import ctypes

    so_path = "/opt/axon/libaxon_pjrt.so"
    lib = ctypes.CDLL(so_path)
    if not hasattr(lib, "axon_start_nrt_profile"):
        return
    lib.axon_start_nrt_profile.argtypes = [
        ctypes.POINTER(ctypes.c_int64), ctypes.c_size_t]
    lib.axon_start_nrt_profile.restype = ctypes.c_int64
    lib.axon_stop_nrt_profile.argtypes = [ctypes.c_char_p]
    lib.axon_stop_nrt_profile.restype = ctypes.c_int64

    @contextlib.contextmanager
    def _hook(output_dir, device_ids):
        import jax

        jax.devices()
        if device_ids:
            ids = (ctypes.c_int64 * len(device_ids))(*device_ids)
            rc = lib.axon_start_nrt_profile(ids, len(device_ids))
        else:
            rc = lib.axon_start_nrt_profile(None, 0)
        if rc != 0:
            raise RuntimeError(f"axon_start_nrt_profile rc={rc}")
        try:
            yield
        finally:
            n = lib.axon_stop_nrt_profile(str(output_dir).encode())
            print(f"ntff profile: {n} file(s) written to {output_dir}",
                  file=sys.stderr)

    axon_hooks.set_axon_ntff_profile_hook(_hook)
